# revision 2
# baseline (speedup 1.0000x reference)
"""MicroExpertMoE Trainium2 kernel.

Computes (per reference):
    temp   = softplus(log_temp) + 0.1
    logits = x @ W_router.T / temp                      [B,T,E]
    thr    = 2nd-largest logit per token (TOP_K=2)
    logits = logits * sigmoid(10*(logits - thr))
    w      = softmax(logits, -1)
    h      = silu(x @ W_gate.T) * (x @ W_up.T)          [B,T,E,K]
    y      = einsum(h * w, W_down)                      [B,T,D]

Sharding: data-parallel over the B*T=8192 tokens across 8 NeuronCores
(1024 tokens/core, all 16 experts/core, no collectives).  All matmuls run
in bfloat16 (1 col/cycle PE rate, same as float32r, but half the DMA and
SBUF traffic).  Weights are pre-transposed and packed partition-major on
the host so every DMA is a contiguous [128, F] copy.

On-chip layout (per core):
    xT      [128p, 8a x 1024t]  d=a*128+p on partitions, resident
    router: logitsT [16,1024] via PE; 8 PE-transposes into one
            [128t, 8x16e] tile; batched top-2 threshold + soft
            suppression + softmax on DVE/ACT (free-dim-broadcast APs)
            -> w_all [128t, 8x16e] resident
    expert loop e=0..15 (weights 4-deep prefetched, chunk-pipelined:
    stage2+down of chunk c emitted after gate/up of chunk c+1):
      gate/up:  hT[k,t] += Wg_T[d,k].T @ xT[d,t]   (PSUM, 8 d-tiles)
      stage2:   hw = silu(g) * u -> bf16           (ACT+DVE)
      down:     y_ps[t,d512] += hw[k,t].T @ Wd_T[k,d] (PSUM halves,
                4-deep so PE never waits on the DVE drain)
      y_sbuf[t,d] (+)= w_all[t,e] * y_ps           (DVE, router weight
                applied post-matmul: (h*w)@Wd == w*(h@Wd))
"""

import numpy as np
import ml_dtypes

import concourse.bacc as bacc
import concourse.tile as tile
import concourse.mybir as mybir
from concourse import masks
from concourse.bass_utils import run_bass_kernel_spmd

B, T, D, E, K = 4, 2048, 1024, 16, 256
NCORES = 8
TOK = B * T // NCORES          # 1024 tokens per core
NT = TOK // 128                # 8 token tiles of 128
ND = D // 128                  # 8 d tiles
NK = K // 128                  # 2 k tiles
TC = 256                       # token chunk for expert stages
NC_CHUNK = TOK // TC           # 4 chunks
dt = mybir.dt
AF = mybir.ActivationFunctionType
ALU = mybir.AluOpType

_built = None


def _emit_body(nc, pools, dram, rep):
    res, rsb, wpool, s2p, psA, psY = pools
    xt, wr, ident, y_sb = dram["xt_t"], dram["wr_t"], dram["ident"], dram["y_sb"]
    wg_d, wu_d, wd_d = dram["wg_d"], dram["wu_d"], dram["wd_d"]

    # ---------- router (batched) ----------
    lgT = rsb.tile([16, TOK], dt.float32, tag="lgT")
    for h in range(2):  # two 512-token halves
        lg_ps = psA.tile([16, 512], dt.float32, tag="gu")
        for a in range(ND):
            nc.tensor.matmul(
                lg_ps[:],
                wr[:, a * E:(a + 1) * E],
                xt[:, a * TOK + h * 512: a * TOK + (h + 1) * 512],
                start=(a == 0), stop=(a == ND - 1),
            )
        nc.vector.tensor_copy(lgT[:, h * 512:(h + 1) * 512], lg_ps[:])

    # transpose all 8 [16,128] slices into one [128, 8*16] tile
    ltr_ps = psA.tile([128, NT * E], dt.float32, tag="gu")
    for tt in range(NT):
        nc.tensor.transpose(
            ltr_ps[:, tt * E:(tt + 1) * E],
            lgT[:, tt * 128:(tt + 1) * 128], ident[:16, :16])
    lg = rsb.tile([128, NT * E], dt.float32, tag="lg")
    nc.vector.tensor_copy(lg[:], ltr_ps[:])

    def v3(ap):  # [128, 8*16] -> [128, 8, 16]
        return ap.rearrange("p (a e) -> p a e", e=E)

    def bc(ap):  # [128, 8] -> [128, 8, 16] broadcast
        return ap.unsqueeze(2).broadcast_to([128, NT, E])

    m1 = rsb.tile([128, NT], dt.float32, tag="m1")
    nc.vector.reduce_max(m1[:], v3(lg[:]), axis=mybir.AxisListType.X)
    dd = rsb.tile([128, NT * E], dt.float32, tag="dd")
    nc.vector.tensor_sub(v3(dd[:]), v3(lg[:]), bc(m1[:]))
    eq = rsb.tile([128, NT * E], dt.float32, tag="eq")
    nc.vector.tensor_scalar(eq[:], dd[:], 0.0, None, op0=ALU.is_ge)
    msk = rsb.tile([128, NT * E], dt.float32, tag="msk")
    nc.vector.scalar_tensor_tensor(msk[:], eq[:], -1e30, lg[:],
                                   op0=ALU.mult, op1=ALU.add)
    thr = rsb.tile([128, NT], dt.float32, tag="thr")
    nc.vector.reduce_max(thr[:], v3(msk[:]), axis=mybir.AxisListType.X)
    sarg = rsb.tile([128, NT * E], dt.float32, tag="sarg")
    nc.vector.tensor_sub(v3(sarg[:]), v3(lg[:]), bc(thr[:]))
    sg = rsb.tile([128, NT * E], dt.float32, tag="sg")
    nc.scalar.activation(sg[:], sarg[:], AF.Sigmoid, scale=10.0)
    l2 = rsb.tile([128, NT * E], dt.float32, tag="l2")
    nc.vector.tensor_mul(l2[:], lg[:], sg[:])
    mx = rsb.tile([128, NT], dt.float32, tag="mx")
    nc.vector.reduce_max(mx[:], v3(l2[:]), axis=mybir.AxisListType.X)
    earg = rsb.tile([128, NT * E], dt.float32, tag="earg")
    nc.vector.tensor_sub(v3(earg[:]), v3(l2[:]), bc(mx[:]))
    ex = rsb.tile([128, NT * E], dt.float32, tag="ex")
    nc.scalar.activation(ex[:], earg[:], AF.Exp)
    sm = rsb.tile([128, NT], dt.float32, tag="sm")
    nc.vector.reduce_sum(sm[:], v3(ex[:]), axis=mybir.AxisListType.X)
    rs = rsb.tile([128, NT], dt.float32, tag="rs")
    nc.vector.reciprocal(rs[:], sm[:])
    # w_all[t, tt*16+e] = softmax weight; resident for the expert loop
    w_all = rsb.tile([128, NT * E], dt.float32, tag="w_all")
    nc.vector.tensor_mul(v3(w_all[:]), v3(ex[:]), bc(rs[:]))

    # ---------- expert loop (software-pipelined) ----------
    def emit_gateup(wg_t, wu_t, c):
        g_ps = psA.tile([128, 2 * TC], dt.float32, tag="gu", name="g_ps")
        u_ps = psA.tile([128, 2 * TC], dt.float32, tag="gu", name="u_ps")
        for w_t, o_ps in ((wg_t, g_ps), (wu_t, u_ps)):
            for kt in range(NK):
                for a in range(ND):
                    nc.tensor.matmul(
                        o_ps[:, kt * TC:(kt + 1) * TC],
                        w_t[:, a * K + kt * 128: a * K + (kt + 1) * 128],
                        xt[:, a * TOK + c * TC: a * TOK + (c + 1) * TC],
                        start=(a == 0), stop=(a == ND - 1),
                    )
        return g_ps, u_ps

    def emit_tail(e, c, g_ps, u_ps, wd_t):
        # stage 2: hw = silu(g) * u  (bf16 stationary for the down matmul)
        sg_t = s2p.tile([128, 2 * TC], dt.float32, tag="sgt", name="sg_t")
        nc.scalar.activation(sg_t[:], g_ps[:], AF.Silu)
        hw_t = s2p.tile([128, 2 * TC], dt.bfloat16, tag="hwt", name="hw_t")
        nc.vector.tensor_mul(hw_t[:], u_ps[:], sg_t[:])

        # down: y[t,d] (+)= w[t,e] * (hw[k,t].T @ WdT[k,d])
        for tt in range(TC // 128):
            gt = c * (TC // 128) + tt      # global token tile
            ys = y_sb[gt]
            w_col = w_all[:, gt * E + e: gt * E + e + 1]
            for dk in range(D // 512):
                y_ps = psY.tile([128, 512], dt.float32, tag="y", name="y_ps")
                for kt in range(NK):
                    nc.tensor.matmul(
                        y_ps[:],
                        hw_t[:, kt * TC + tt * 128: kt * TC + (tt + 1) * 128],
                        wd_t[:, kt * D + dk * 512: kt * D + (dk + 1) * 512],
                        start=(kt == 0), stop=(kt == NK - 1),
                    )
                yd = ys[:, dk * 512:(dk + 1) * 512]
                if e == 0:
                    nc.vector.tensor_scalar(yd, y_ps[:], w_col, None,
                                            op0=ALU.mult)
                else:
                    nc.vector.scalar_tensor_tensor(yd, y_ps[:], w_col, yd,
                                                   op0=ALU.mult, op1=ALU.add)

    pending = None
    for e in range(E):
        wg_t = wpool.tile([128, ND * K], dt.bfloat16, tag="wg", name="wg_t")
        nc.sync.dma_start(wg_t[:], wg_d[e])
        wu_t = wpool.tile([128, ND * K], dt.bfloat16, tag="wu", name="wu_t")
        nc.sync.dma_start(wu_t[:], wu_d[e])
        wd_t = wpool.tile([128, NK * D], dt.bfloat16, tag="wd", name="wd_t")
        nc.sync.dma_start(wd_t[:], wd_d[e])

        for c in range(NC_CHUNK):
            g_ps, u_ps = emit_gateup(wg_t, wu_t, c)
            if pending is not None:
                emit_tail(*pending)
            pending = (e, c, g_ps, u_ps, wd_t)
    emit_tail(*pending)


def _build(repeat=1):
    nc = bacc.Bacc("TRN2", target_bir_lowering=False, debug=False,
                   num_devices=NCORES)

    xt_d = nc.dram_tensor("xt", (128, ND * TOK), dt.bfloat16,
                          kind="ExternalInput").ap()
    wg_d = nc.dram_tensor("wg", (E, 128, ND * K), dt.bfloat16,
                          kind="ExternalInput").ap()
    wu_d = nc.dram_tensor("wu", (E, 128, ND * K), dt.bfloat16,
                          kind="ExternalInput").ap()
    wd_d = nc.dram_tensor("wd", (E, 128, NK * D), dt.bfloat16,
                          kind="ExternalInput").ap()
    wr_d = nc.dram_tensor("wr", (128, ND * E), dt.bfloat16,
                          kind="ExternalInput").ap()
    y_d = nc.dram_tensor("y", (TOK, D), dt.float32,
                         kind="ExternalOutput").ap()

    with tile.TileContext(nc) as tc:
        with (
            tc.tile_pool(name="resident", bufs=1) as res,
            tc.tile_pool(name="router_sb", bufs=2) as rsb,
            tc.tile_pool(name="wpool", bufs=4) as wpool,
            tc.tile_pool(name="stage2", bufs=3) as s2p,
            tc.tile_pool(name="psA", bufs=4, space="PSUM") as psA,
            tc.tile_pool(name="psY", bufs=4, space="PSUM") as psY,
        ):
            # ---------- resident loads ----------
            wr = res.tile([128, ND * E], dt.bfloat16, tag="wr")
            nc.sync.dma_start(wr[:], wr_d)
            xt = res.tile([128, ND * TOK], dt.bfloat16, tag="xt")
            for a in range(ND):
                nc.sync.dma_start(xt[:, a * TOK:(a + 1) * TOK],
                                  xt_d[:, a * TOK:(a + 1) * TOK])
            ident = res.tile([128, 128], dt.float32, tag="ident")
            masks.make_identity(nc, ident[:])
            y_sb = [res.tile([128, D], dt.float32, tag=f"ysb{i}",
                             name=f"ysb{i}") for i in range(NT)]

            pools = (res, rsb, wpool, s2p, psA, psY)
            dram = dict(xt_t=xt, wr_t=wr, ident=ident, y_sb=y_sb,
                        wg_d=wg_d, wu_d=wu_d, wd_d=wd_d)
            for _ in range(repeat):
                _emit_body(nc, pools, dram, _)

            # ---------- store ----------
            for i in range(NT):
                nc.sync.dma_start(y_d[i * 128:(i + 1) * 128, :], y_sb[i][:])

    nc.compile()
    return nc


def _prep_inputs(x, W_up, W_gate, W_down, W_router, log_temp):
    """Host-side repack: fold temp, transpose weights, partition-major."""
    bf16 = ml_dtypes.bfloat16
    x = np.asarray(x, dtype=np.float32)
    W_up = np.asarray(W_up, dtype=np.float32)
    W_gate = np.asarray(W_gate, dtype=np.float32)
    W_down = np.asarray(W_down, dtype=np.float32)
    W_router = np.asarray(W_router, dtype=np.float32)
    lt = float(np.asarray(log_temp, dtype=np.float32))
    temp = float(np.log1p(np.exp(lt)) + 0.1)

    X = x.reshape(B * T, D)
    xT = np.ascontiguousarray(X.T)                       # [D, 8192]
    xt_cores = []
    for c in range(NCORES):
        sl = xT[:, c * TOK:(c + 1) * TOK]                # [1024, 1024]
        xt_cores.append(np.ascontiguousarray(
            sl.reshape(ND, 128, TOK).transpose(1, 0, 2)
              .reshape(128, ND * TOK)).astype(bf16))

    def pack_dk(w):  # [E, K, D] (torch [out,in]) -> [E, 128, 8*K], d-major
        wt = w.transpose(0, 2, 1)                        # [E, D, K]
        return np.ascontiguousarray(
            wt.reshape(E, ND, 128, K).transpose(0, 2, 1, 3)
              .reshape(E, 128, ND * K)).astype(bf16)

    wg_h = pack_dk(W_gate)
    wu_h = pack_dk(W_up)
    wdt = W_down.transpose(0, 2, 1)                      # [E, K, D]
    wd_h = np.ascontiguousarray(
        wdt.reshape(E, NK, 128, D).transpose(0, 2, 1, 3)
           .reshape(E, 128, NK * D)).astype(bf16)
    wrt = np.ascontiguousarray(W_router.T) / temp        # [D, E]
    wr_h = np.ascontiguousarray(
        wrt.reshape(ND, 128, E).transpose(1, 0, 2)
           .reshape(128, ND * E)).astype(bf16)

    in_maps = []
    for c in range(NCORES):
        in_maps.append({
            "xt": xt_cores[c],
            "wg": wg_h, "wu": wu_h, "wd": wd_h, "wr": wr_h,
        })
    return in_maps


def kernel(x, W_up, W_gate, W_down, W_router, log_temp, _trace=False):
    global _built
    if _built is None:
        _built = _build()
    nc = _built
    in_maps = _prep_inputs(x, W_up, W_gate, W_down, W_router, log_temp)
    res = run_bass_kernel_spmd(nc, in_maps, core_ids=list(range(NCORES)),
                               trace=_trace)
    out = np.empty((B * T, D), dtype=np.float32)
    for c in range(NCORES):
        out[c * TOK:(c + 1) * TOK, :] = res.results[c]["y"]
    kernel.last_results = res
    return out.reshape(B, T, D)


# revision 6
# speedup vs baseline: 1.0477x; 1.0477x over previous
"""MicroExpertMoE Trainium2 kernel.

Computes (per reference):
    temp   = softplus(log_temp) + 0.1
    logits = x @ W_router.T / temp                      [B,T,E]
    thr    = 2nd-largest logit per token (TOP_K=2)
    logits = logits * sigmoid(10*(logits - thr))
    w      = softmax(logits, -1)
    h      = silu(x @ W_gate.T) * (x @ W_up.T)          [B,T,E,K]
    y      = einsum(h * w, W_down)                      [B,T,D]

Sharding: data-parallel over the B*T=8192 tokens across 8 NeuronCores
(1024 tokens/core, all 16 experts/core, no collectives).  All matmuls run
in bfloat16 (1 col/cycle PE rate, same as float32r, but half the DMA and
SBUF traffic).  Weights are pre-transposed and packed partition-major on
the host so every DMA is a contiguous [128, F] copy.

On-chip layout (per core):
    xT      [128p, 8a x 1024t]  d=a*128+p on partitions, resident
    router: logitsT [16,1024] via PE; 8 PE-transposes into one
            [128t, 8x16e] tile; batched top-2 threshold + soft
            suppression + softmax on DVE/ACT (free-dim-broadcast APs)
            -> w_all [128t, 8x16e] resident
    expert loop e=0..15 (weights 4-deep prefetched, chunk-pipelined:
    stage2+down of chunk c emitted after gate/up of chunk c+1):
      gate/up:  hT[k,t] += Wg_T[d,k].T @ xT[d,t]   (PSUM, 8 d-tiles)
      stage2:   hw = silu(g) * u -> bf16           (ACT+DVE)
      down:     y_ps[t,d512] += hw[k,t].T @ Wd_T[k,d] (PSUM halves,
                4-deep so PE never waits on the DVE drain)
      y_sbuf[t,d] (+)= w_all[t,e] * y_ps           (DVE, router weight
                applied post-matmul: (h*w)@Wd == w*(h@Wd))
"""

import numpy as np
import ml_dtypes

import concourse.bacc as bacc
import concourse.tile as tile
import concourse.mybir as mybir
from concourse import masks
from concourse.bass_utils import run_bass_kernel_spmd

B, T, D, E, K = 4, 2048, 1024, 16, 256
NCORES = 8
TOK = B * T // NCORES          # 1024 tokens per core
NT = TOK // 128                # 8 token tiles of 128
ND = D // 128                  # 8 d tiles
NK = K // 128                  # 2 k tiles
TC = 256                       # token chunk for expert stages
NC_CHUNK = TOK // TC           # 4 chunks
dt = mybir.dt
AF = mybir.ActivationFunctionType
ALU = mybir.AluOpType

_built = None


def _emit_body(nc, pools, dram, rep):
    res, rsb, wpool, s2p, psA, psY = pools
    xt, wr, ident, y_sb = dram["xt_t"], dram["wr_t"], dram["ident"], dram["y_sb"]
    wg_d, wu_d, wd_d = dram["wg_d"], dram["wu_d"], dram["wd_d"]

    # ---------- router (batched) ----------
    lgT = rsb.tile([16, TOK], dt.float32, tag="lgT")
    for h in range(2):  # two 512-token halves
        lg_ps = psA.tile([16, 512], dt.float32, tag="gu")
        for a in range(ND):
            nc.tensor.matmul(
                lg_ps[:],
                wr[:, a * E:(a + 1) * E],
                xt[:, a * TOK + h * 512: a * TOK + (h + 1) * 512],
                start=(a == 0), stop=(a == ND - 1),
            )
        nc.vector.tensor_copy(lgT[:, h * 512:(h + 1) * 512], lg_ps[:])

    # transpose all 8 [16,128] slices into one [128, 8*16] tile
    ltr_ps = psA.tile([128, NT * E], dt.float32, tag="gu")
    for tt in range(NT):
        nc.tensor.transpose(
            ltr_ps[:, tt * E:(tt + 1) * E],
            lgT[:, tt * 128:(tt + 1) * 128], ident[:16, :16])
    lg = rsb.tile([128, NT * E], dt.float32, tag="lg")
    nc.vector.tensor_copy(lg[:], ltr_ps[:])

    def v3(ap):  # [128, 8*16] -> [128, 8, 16]
        return ap.rearrange("p (a e) -> p a e", e=E)

    def bc(ap):  # [128, 8] -> [128, 8, 16] broadcast
        return ap.unsqueeze(2).broadcast_to([128, NT, E])

    m1 = rsb.tile([128, NT], dt.float32, tag="m1")
    nc.vector.reduce_max(m1[:], v3(lg[:]), axis=mybir.AxisListType.X)
    dd = rsb.tile([128, NT * E], dt.float32, tag="dd")
    nc.vector.tensor_sub(v3(dd[:]), v3(lg[:]), bc(m1[:]))
    eq = rsb.tile([128, NT * E], dt.float32, tag="eq")
    nc.vector.tensor_scalar(eq[:], dd[:], 0.0, None, op0=ALU.is_ge)
    msk = rsb.tile([128, NT * E], dt.float32, tag="msk")
    nc.vector.scalar_tensor_tensor(msk[:], eq[:], -1e30, lg[:],
                                   op0=ALU.mult, op1=ALU.add)
    thr = rsb.tile([128, NT], dt.float32, tag="thr")
    nc.vector.reduce_max(thr[:], v3(msk[:]), axis=mybir.AxisListType.X)
    sarg = rsb.tile([128, NT * E], dt.float32, tag="sarg")
    nc.vector.tensor_sub(v3(sarg[:]), v3(lg[:]), bc(thr[:]))
    sg = rsb.tile([128, NT * E], dt.float32, tag="sg")
    nc.scalar.activation(sg[:], sarg[:], AF.Sigmoid, scale=10.0)
    l2 = rsb.tile([128, NT * E], dt.float32, tag="l2")
    nc.vector.tensor_mul(l2[:], lg[:], sg[:])
    mx = rsb.tile([128, NT], dt.float32, tag="mx")
    nc.vector.reduce_max(mx[:], v3(l2[:]), axis=mybir.AxisListType.X)
    earg = rsb.tile([128, NT * E], dt.float32, tag="earg")
    nc.vector.tensor_sub(v3(earg[:]), v3(l2[:]), bc(mx[:]))
    ex = rsb.tile([128, NT * E], dt.float32, tag="ex")
    nc.scalar.activation(ex[:], earg[:], AF.Exp)
    sm = rsb.tile([128, NT], dt.float32, tag="sm")
    nc.vector.reduce_sum(sm[:], v3(ex[:]), axis=mybir.AxisListType.X)
    rs = rsb.tile([128, NT], dt.float32, tag="rs")
    nc.vector.reciprocal(rs[:], sm[:])
    # w_all[t, tt*16+e] = softmax weight; resident for the expert loop
    w_all = rsb.tile([128, NT * E], dt.float32, tag="w_all")
    nc.vector.tensor_mul(v3(w_all[:]), v3(ex[:]), bc(rs[:]))

    # ---------- expert loop (software-pipelined) ----------
    def emit_gateup(wg_t, wu_t, c):
        g_ps = psA.tile([128, 2 * TC], dt.float32, tag="gu", name="g_ps")
        u_ps = psA.tile([128, 2 * TC], dt.float32, tag="gu", name="u_ps")
        for w_t, o_ps in ((wg_t, g_ps), (wu_t, u_ps)):
            for kt in range(NK):
                for a in range(ND):
                    nc.tensor.matmul(
                        o_ps[:, kt * TC:(kt + 1) * TC],
                        w_t[:, a * K + kt * 128: a * K + (kt + 1) * 128],
                        xt[:, a * TOK + c * TC: a * TOK + (c + 1) * TC],
                        start=(a == 0), stop=(a == ND - 1),
                    )
        return g_ps, u_ps

    def emit_tail(e, c, g_ps, u_ps, wd_t):
        # stage 2: hw = silu(g) * u  (bf16 stationary for the down matmul)
        sg_t = s2p.tile([128, 2 * TC], dt.float32, tag="sgt", name="sg_t")
        nc.scalar.activation(sg_t[:], g_ps[:], AF.Silu)
        hw_t = s2p.tile([128, 2 * TC], dt.bfloat16, tag="hwt", name="hw_t")
        nc.vector.tensor_mul(hw_t[:], u_ps[:], sg_t[:])

        # down: y[t,d] (+)= w[t,e] * (hw[k,t].T @ WdT[k,d])
        for tt in range(TC // 128):
            gt = c * (TC // 128) + tt      # global token tile
            ys = y_sb[gt]
            w_col = w_all[:, gt * E + e: gt * E + e + 1]
            for dk in range(D // 512):
                y_ps = psY.tile([128, 512], dt.float32, tag="y", name="y_ps")
                for kt in range(NK):
                    nc.tensor.matmul(
                        y_ps[:],
                        hw_t[:, kt * TC + tt * 128: kt * TC + (tt + 1) * 128],
                        wd_t[:, kt * D + dk * 512: kt * D + (dk + 1) * 512],
                        start=(kt == 0), stop=(kt == NK - 1),
                    )
                yd = ys[:, dk * 512:(dk + 1) * 512]
                if e == 0:
                    nc.vector.tensor_scalar(yd, y_ps[:], w_col, None,
                                            op0=ALU.mult)
                else:
                    nc.vector.scalar_tensor_tensor(yd, y_ps[:], w_col, yd,
                                                   op0=ALU.mult, op1=ALU.add)

    pending = None
    for e in range(E):
        wg_t = wpool.tile([128, ND * K], dt.bfloat16, tag="wg", name="wg_t")
        nc.sync.dma_start(wg_t[:], wg_d[e])
        wu_t = wpool.tile([128, ND * K], dt.bfloat16, tag="wu", name="wu_t")
        nc.sync.dma_start(wu_t[:], wu_d[e])
        wd_t = wpool.tile([128, NK * D], dt.bfloat16, tag="wd", name="wd_t")
        nc.sync.dma_start(wd_t[:], wd_d[e])

        for c in range(NC_CHUNK):
            g_ps, u_ps = emit_gateup(wg_t, wu_t, c)
            if pending is not None:
                emit_tail(*pending)
            pending = (e, c, g_ps, u_ps, wd_t)
    emit_tail(*pending)


def _build(repeat=1):
    nc = bacc.Bacc("TRN2", target_bir_lowering=False, debug=False,
                   num_devices=NCORES)

    xt_d = nc.dram_tensor("xt", (128, ND * TOK), dt.bfloat16,
                          kind="ExternalInput").ap()
    wg_d = nc.dram_tensor("wg", (E, 128, ND * K), dt.bfloat16,
                          kind="ExternalInput").ap()
    wu_d = nc.dram_tensor("wu", (E, 128, ND * K), dt.bfloat16,
                          kind="ExternalInput").ap()
    wd_d = nc.dram_tensor("wd", (E, 128, NK * D), dt.bfloat16,
                          kind="ExternalInput").ap()
    wr_d = nc.dram_tensor("wr", (128, ND * E), dt.bfloat16,
                          kind="ExternalInput").ap()
    y_d = nc.dram_tensor("y", (TOK, D), dt.float32,
                         kind="ExternalOutput").ap()

    with tile.TileContext(nc) as tc:
        with (
            tc.tile_pool(name="resident", bufs=1) as res,
            tc.tile_pool(name="router_sb", bufs=2) as rsb,
            tc.tile_pool(name="wpool", bufs=4) as wpool,
            tc.tile_pool(name="stage2", bufs=3) as s2p,
            tc.tile_pool(name="psA", bufs=4, space="PSUM") as psA,
            tc.tile_pool(name="psY", bufs=4, space="PSUM") as psY,
        ):
            # ---------- resident loads ----------
            wr = res.tile([128, ND * E], dt.bfloat16, tag="wr")
            nc.sync.dma_start(wr[:], wr_d)
            xt = res.tile([128, ND * TOK], dt.bfloat16, tag="xt")
            for a in range(ND):
                nc.sync.dma_start(xt[:, a * TOK:(a + 1) * TOK],
                                  xt_d[:, a * TOK:(a + 1) * TOK])
            ident = res.tile([128, 128], dt.float32, tag="ident")
            masks.make_identity(nc, ident[:])
            y_sb = [res.tile([128, D], dt.float32, tag=f"ysb{i}",
                             name=f"ysb{i}") for i in range(NT)]

            pools = (res, rsb, wpool, s2p, psA, psY)
            dram = dict(xt_t=xt, wr_t=wr, ident=ident, y_sb=y_sb,
                        wg_d=wg_d, wu_d=wu_d, wd_d=wd_d)
            for _ in range(repeat):
                _emit_body(nc, pools, dram, _)

            # ---------- store ----------
            for i in range(NT):
                nc.sync.dma_start(y_d[i * 128:(i + 1) * 128, :], y_sb[i][:])

    nc.compile()
    return nc


def _prep_inputs(x, W_up, W_gate, W_down, W_router, log_temp):
    """Host-side repack: fold temp, transpose weights, partition-major."""
    bf16 = ml_dtypes.bfloat16
    x = np.asarray(x, dtype=np.float32)
    W_up = np.asarray(W_up, dtype=np.float32)
    W_gate = np.asarray(W_gate, dtype=np.float32)
    W_down = np.asarray(W_down, dtype=np.float32)
    W_router = np.asarray(W_router, dtype=np.float32)
    lt = float(np.asarray(log_temp, dtype=np.float32))
    temp = float(np.log1p(np.exp(lt)) + 0.1)

    X = x.reshape(B * T, D)
    xT = np.ascontiguousarray(X.T)                       # [D, 8192]
    xt_cores = []
    for c in range(NCORES):
        sl = xT[:, c * TOK:(c + 1) * TOK]                # [1024, 1024]
        xt_cores.append(np.ascontiguousarray(
            sl.reshape(ND, 128, TOK).transpose(1, 0, 2)
              .reshape(128, ND * TOK)).astype(bf16))

    def pack_dk(w):  # [E, K, D] (torch [out,in]) -> [E, 128, 8*K], d-major
        wt = w.transpose(0, 2, 1)                        # [E, D, K]
        return np.ascontiguousarray(
            wt.reshape(E, ND, 128, K).transpose(0, 2, 1, 3)
              .reshape(E, 128, ND * K)).astype(bf16)

    wg_h = pack_dk(W_gate)
    wu_h = pack_dk(W_up)
    wdt = W_down.transpose(0, 2, 1)                      # [E, K, D]
    wd_h = np.ascontiguousarray(
        wdt.reshape(E, NK, 128, D).transpose(0, 2, 1, 3)
           .reshape(E, 128, NK * D)).astype(bf16)
    wrt = np.ascontiguousarray(W_router.T) / temp        # [D, E]
    wr_h = np.ascontiguousarray(
        wrt.reshape(ND, 128, E).transpose(1, 0, 2)
           .reshape(128, ND * E)).astype(bf16)

    in_maps = []
    for c in range(NCORES):
        in_maps.append({
            "xt": xt_cores[c],
            "wg": wg_h, "wu": wu_h, "wd": wd_h, "wr": wr_h,
        })
    return in_maps


def kernel(x, W_up, W_gate, W_down, W_router, log_temp, _trace=False):
    global _built
    if _built is None:
        _built = _build()
    nc = _built
    in_maps = _prep_inputs(x, W_up, W_gate, W_down, W_router, log_temp)
    res = run_bass_kernel_spmd(nc, in_maps, core_ids=list(range(NCORES)),
                               trace=_trace)
    out = np.empty((B * T, D), dtype=np.float32)
    for c in range(NCORES):
        out[c * TOK:(c + 1) * TOK, :] = res.results[c]["y"]
    kernel.last_results = res
    return out.reshape(B, T, D)


# revision 7
# speedup vs baseline: 1.0831x; 1.0337x over previous
"""MicroExpertMoE Trainium2 kernel.

Computes (per reference):
    temp   = softplus(log_temp) + 0.1
    logits = x @ W_router.T / temp                      [B,T,E]
    thr    = 2nd-largest logit per token (TOP_K=2)
    logits = logits * sigmoid(10*(logits - thr))
    w      = softmax(logits, -1)
    h      = silu(x @ W_gate.T) * (x @ W_up.T)          [B,T,E,K]
    y      = einsum(h * w, W_down)                      [B,T,D]

Sharding: data-parallel over the B*T=8192 tokens across 8 NeuronCores
(1024 tokens/core, all 16 experts/core, no collectives).  All matmuls run
in bfloat16 (1 col/cycle PE rate, same as float32r, but half the DMA and
SBUF traffic).  Weights are pre-transposed and packed partition-major on
the host so every DMA is a contiguous [128, F] copy.

On-chip layout (per core):
    xT      [128p, 8a x 1024t]  d=a*128+p on partitions, resident
    router: logitsT [16,1024] via PE; 8 PE-transposes into one
            [128t, 8x16e] tile; batched top-2 threshold + soft
            suppression + softmax on DVE/ACT (free-dim-broadcast APs)
            -> w_all [128t, 8x16e] resident
    expert loop e=0..15 (weights 4-deep prefetched, chunk-pipelined:
    stage2+down of chunk c emitted after gate/up of chunk c+1):
      gate/up:  hT[k,t] += Wg_T[d,k].T @ xT[d,t]   (PSUM, 8 d-tiles)
      stage2:   hw = silu(g) * u -> bf16           (ACT+DVE)
      down:     y_ps[t,d512] += hw[k,t].T @ Wd_T[k,d] (PSUM halves,
                4-deep so PE never waits on the DVE drain)
      y_sbuf[t,d] (+)= w_all[t,e] * y_ps           (DVE, router weight
                applied post-matmul: (h*w)@Wd == w*(h@Wd))
"""

import numpy as np
import ml_dtypes

import concourse.bacc as bacc
import concourse.tile as tile
import concourse.mybir as mybir
from concourse import masks
from concourse.bass_utils import run_bass_kernel_spmd

B, T, D, E, K = 4, 2048, 1024, 16, 256
NCORES = 8
TOK = B * T // NCORES          # 1024 tokens per core
NT = TOK // 128                # 8 token tiles of 128
ND = D // 128                  # 8 d tiles
NK = K // 128                  # 2 k tiles
TC = 256                       # token chunk for expert stages
NC_CHUNK = TOK // TC           # 4 chunks
dt = mybir.dt
AF = mybir.ActivationFunctionType
ALU = mybir.AluOpType

_built = None


def _emit_body(nc, pools, dram, rep):
    res, rsb, wpool, s2p, psA, psY = pools
    xt, wr, ident, y_sb = dram["xt_t"], dram["wr_t"], dram["ident"], dram["y_sb"]
    wg_d, wu_d, wd_d = dram["wg_d"], dram["wu_d"], dram["wd_d"]

    # ---------- router (batched) ----------
    lgT = rsb.tile([16, TOK], dt.float32, tag="lgT")
    for h in range(2):  # two 512-token halves
        lg_ps = psA.tile([16, 512], dt.float32, tag="gu")
        for a in range(ND):
            nc.tensor.matmul(
                lg_ps[:],
                wr[:, a * E:(a + 1) * E],
                xt[:, a * TOK + h * 512: a * TOK + (h + 1) * 512],
                start=(a == 0), stop=(a == ND - 1),
            )
        nc.vector.tensor_copy(lgT[:, h * 512:(h + 1) * 512], lg_ps[:])

    # transpose all 8 [16,128] slices into one [128, 8*16] tile
    ltr_ps = psA.tile([128, NT * E], dt.float32, tag="gu")
    for tt in range(NT):
        nc.tensor.transpose(
            ltr_ps[:, tt * E:(tt + 1) * E],
            lgT[:, tt * 128:(tt + 1) * 128], ident[:16, :16])
    lg = rsb.tile([128, NT * E], dt.float32, tag="lg")
    nc.vector.tensor_copy(lg[:], ltr_ps[:])

    def v3(ap):  # [128, 8*16] -> [128, 8, 16]
        return ap.rearrange("p (a e) -> p a e", e=E)

    def bc(ap):  # [128, 8] -> [128, 8, 16] broadcast
        return ap.unsqueeze(2).broadcast_to([128, NT, E])

    m1 = rsb.tile([128, NT], dt.float32, tag="m1")
    nc.vector.reduce_max(m1[:], v3(lg[:]), axis=mybir.AxisListType.X)
    dd = rsb.tile([128, NT * E], dt.float32, tag="dd")
    nc.vector.tensor_sub(v3(dd[:]), v3(lg[:]), bc(m1[:]))
    eq = rsb.tile([128, NT * E], dt.float32, tag="eq")
    nc.vector.tensor_scalar(eq[:], dd[:], 0.0, None, op0=ALU.is_ge)
    msk = rsb.tile([128, NT * E], dt.float32, tag="msk")
    nc.vector.scalar_tensor_tensor(msk[:], eq[:], -1e30, lg[:],
                                   op0=ALU.mult, op1=ALU.add)
    thr = rsb.tile([128, NT], dt.float32, tag="thr")
    nc.vector.reduce_max(thr[:], v3(msk[:]), axis=mybir.AxisListType.X)
    sarg = rsb.tile([128, NT * E], dt.float32, tag="sarg")
    nc.vector.tensor_sub(v3(sarg[:]), v3(lg[:]), bc(thr[:]))
    sg = rsb.tile([128, NT * E], dt.float32, tag="sg")
    nc.scalar.activation(sg[:], sarg[:], AF.Sigmoid, scale=10.0)
    l2 = rsb.tile([128, NT * E], dt.float32, tag="l2")
    nc.vector.tensor_mul(l2[:], lg[:], sg[:])
    mx = rsb.tile([128, NT], dt.float32, tag="mx")
    nc.vector.reduce_max(mx[:], v3(l2[:]), axis=mybir.AxisListType.X)
    earg = rsb.tile([128, NT * E], dt.float32, tag="earg")
    nc.vector.tensor_sub(v3(earg[:]), v3(l2[:]), bc(mx[:]))
    ex = rsb.tile([128, NT * E], dt.float32, tag="ex")
    nc.scalar.activation(ex[:], earg[:], AF.Exp)
    sm = rsb.tile([128, NT], dt.float32, tag="sm")
    nc.vector.reduce_sum(sm[:], v3(ex[:]), axis=mybir.AxisListType.X)
    rs = rsb.tile([128, NT], dt.float32, tag="rs")
    nc.vector.reciprocal(rs[:], sm[:])
    # w_all[t, tt*16+e] = softmax weight; resident for the expert loop
    w_all = rsb.tile([128, NT * E], dt.float32, tag="w_all")
    nc.vector.tensor_mul(v3(w_all[:]), v3(ex[:]), bc(rs[:]))

    # ---------- expert loop (software-pipelined) ----------
    def emit_gateup(wg_t, wu_t, c):
        g_ps = psA.tile([128, 2 * TC], dt.float32, tag="gu", name="g_ps")
        u_ps = psA.tile([128, 2 * TC], dt.float32, tag="gu", name="u_ps")
        for w_t, o_ps in ((wg_t, g_ps), (wu_t, u_ps)):
            for kt in range(NK):
                for a in range(ND):
                    nc.tensor.matmul(
                        o_ps[:, kt * TC:(kt + 1) * TC],
                        w_t[:, a * K + kt * 128: a * K + (kt + 1) * 128],
                        xt[:, a * TOK + c * TC: a * TOK + (c + 1) * TC],
                        start=(a == 0), stop=(a == ND - 1),
                    )
        return g_ps, u_ps

    def emit_tail(e, c, g_ps, u_ps, wd_t):
        # stage 2: hw = silu(g) * u  (bf16 stationary for the down matmul)
        sg_t = s2p.tile([128, 2 * TC], dt.float32, tag="sgt", name="sg_t")
        nc.scalar.activation(sg_t[:], g_ps[:], AF.Silu)
        hw_t = s2p.tile([128, 2 * TC], dt.bfloat16, tag="hwt", name="hw_t")
        nc.vector.tensor_mul(hw_t[:], u_ps[:], sg_t[:])

        # down: y[t,d] (+)= w[t,e] * (hw[k,t].T @ WdT[k,d])
        for tt in range(TC // 128):
            gt = c * (TC // 128) + tt      # global token tile
            ys = y_sb[gt]
            w_col = w_all[:, gt * E + e: gt * E + e + 1]
            for dk in range(D // 512):
                y_ps = psY.tile([128, 512], dt.float32, tag="y", name="y_ps")
                for kt in range(NK):
                    nc.tensor.matmul(
                        y_ps[:],
                        hw_t[:, kt * TC + tt * 128: kt * TC + (tt + 1) * 128],
                        wd_t[:, kt * D + dk * 512: kt * D + (dk + 1) * 512],
                        start=(kt == 0), stop=(kt == NK - 1),
                    )
                yd = ys[:, dk * 512:(dk + 1) * 512]
                if e == 0:
                    nc.vector.tensor_scalar(yd, y_ps[:], w_col, None,
                                            op0=ALU.mult)
                else:
                    nc.vector.scalar_tensor_tensor(yd, y_ps[:], w_col, yd,
                                                   op0=ALU.mult, op1=ALU.add)

    pending = None
    for e in range(E):
        wg_t = wpool.tile([128, ND * K], dt.bfloat16, tag="wg", name="wg_t")
        nc.sync.dma_start(wg_t[:], wg_d[e])
        wu_t = wpool.tile([128, ND * K], dt.bfloat16, tag="wu", name="wu_t")
        nc.sync.dma_start(wu_t[:], wu_d[e])
        wd_t = wpool.tile([128, NK * D], dt.bfloat16, tag="wd", name="wd_t")
        nc.sync.dma_start(wd_t[:], wd_d[e])

        for c in range(NC_CHUNK):
            g_ps, u_ps = emit_gateup(wg_t, wu_t, c)
            if pending is not None:
                emit_tail(*pending)
            pending = (e, c, g_ps, u_ps, wd_t)
    emit_tail(*pending)


def _build(repeat=1):
    nc = bacc.Bacc("TRN2", target_bir_lowering=False, debug=False,
                   num_devices=NCORES)

    xt_d = nc.dram_tensor("xt", (128, ND * TOK), dt.bfloat16,
                          kind="ExternalInput").ap()
    wg_d = nc.dram_tensor("wg", (E, 128, ND * K), dt.bfloat16,
                          kind="ExternalInput").ap()
    wu_d = nc.dram_tensor("wu", (E, 128, ND * K), dt.bfloat16,
                          kind="ExternalInput").ap()
    wd_d = nc.dram_tensor("wd", (E, 128, NK * D), dt.bfloat16,
                          kind="ExternalInput").ap()
    wr_d = nc.dram_tensor("wr", (128, ND * E), dt.bfloat16,
                          kind="ExternalInput").ap()
    y_d = nc.dram_tensor("y", (TOK, D), dt.float32,
                         kind="ExternalOutput").ap()

    with tile.TileContext(nc) as tc:
        with (
            tc.tile_pool(name="resident", bufs=1) as res,
            tc.tile_pool(name="router_sb", bufs=2) as rsb,
            tc.tile_pool(name="wpool", bufs=6) as wpool,
            tc.tile_pool(name="stage2", bufs=3) as s2p,
            tc.tile_pool(name="psA", bufs=4, space="PSUM") as psA,
            tc.tile_pool(name="psY", bufs=4, space="PSUM") as psY,
        ):
            # ---------- resident loads ----------
            wr = res.tile([128, ND * E], dt.bfloat16, tag="wr")
            nc.sync.dma_start(wr[:], wr_d)
            xt = res.tile([128, ND * TOK], dt.bfloat16, tag="xt")
            for a in range(ND):
                nc.sync.dma_start(xt[:, a * TOK:(a + 1) * TOK],
                                  xt_d[:, a * TOK:(a + 1) * TOK])
            ident = res.tile([128, 128], dt.float32, tag="ident")
            masks.make_identity(nc, ident[:])
            y_sb = [res.tile([128, D], dt.float32, tag=f"ysb{i}",
                             name=f"ysb{i}") for i in range(NT)]

            pools = (res, rsb, wpool, s2p, psA, psY)
            dram = dict(xt_t=xt, wr_t=wr, ident=ident, y_sb=y_sb,
                        wg_d=wg_d, wu_d=wu_d, wd_d=wd_d)
            for _ in range(repeat):
                _emit_body(nc, pools, dram, _)

            # ---------- store ----------
            for i in range(NT):
                nc.sync.dma_start(y_d[i * 128:(i + 1) * 128, :], y_sb[i][:])

    nc.compile()
    return nc


def _prep_inputs(x, W_up, W_gate, W_down, W_router, log_temp):
    """Host-side repack: fold temp, transpose weights, partition-major."""
    bf16 = ml_dtypes.bfloat16
    x = np.asarray(x, dtype=np.float32)
    W_up = np.asarray(W_up, dtype=np.float32)
    W_gate = np.asarray(W_gate, dtype=np.float32)
    W_down = np.asarray(W_down, dtype=np.float32)
    W_router = np.asarray(W_router, dtype=np.float32)
    lt = float(np.asarray(log_temp, dtype=np.float32))
    temp = float(np.log1p(np.exp(lt)) + 0.1)

    X = x.reshape(B * T, D)
    xT = np.ascontiguousarray(X.T)                       # [D, 8192]
    xt_cores = []
    for c in range(NCORES):
        sl = xT[:, c * TOK:(c + 1) * TOK]                # [1024, 1024]
        xt_cores.append(np.ascontiguousarray(
            sl.reshape(ND, 128, TOK).transpose(1, 0, 2)
              .reshape(128, ND * TOK)).astype(bf16))

    def pack_dk(w):  # [E, K, D] (torch [out,in]) -> [E, 128, 8*K], d-major
        wt = w.transpose(0, 2, 1)                        # [E, D, K]
        return np.ascontiguousarray(
            wt.reshape(E, ND, 128, K).transpose(0, 2, 1, 3)
              .reshape(E, 128, ND * K)).astype(bf16)

    wg_h = pack_dk(W_gate)
    wu_h = pack_dk(W_up)
    wdt = W_down.transpose(0, 2, 1)                      # [E, K, D]
    wd_h = np.ascontiguousarray(
        wdt.reshape(E, NK, 128, D).transpose(0, 2, 1, 3)
           .reshape(E, 128, NK * D)).astype(bf16)
    wrt = np.ascontiguousarray(W_router.T) / temp        # [D, E]
    wr_h = np.ascontiguousarray(
        wrt.reshape(ND, 128, E).transpose(1, 0, 2)
           .reshape(128, ND * E)).astype(bf16)

    in_maps = []
    for c in range(NCORES):
        in_maps.append({
            "xt": xt_cores[c],
            "wg": wg_h, "wu": wu_h, "wd": wd_h, "wr": wr_h,
        })
    return in_maps


def kernel(x, W_up, W_gate, W_down, W_router, log_temp, _trace=False):
    global _built
    if _built is None:
        _built = _build()
    nc = _built
    in_maps = _prep_inputs(x, W_up, W_gate, W_down, W_router, log_temp)
    res = run_bass_kernel_spmd(nc, in_maps, core_ids=list(range(NCORES)),
                               trace=_trace)
    out = np.empty((B * T, D), dtype=np.float32)
    for c in range(NCORES):
        out[c * TOK:(c + 1) * TOK, :] = res.results[c]["y"]
    kernel.last_results = res
    return out.reshape(B, T, D)
